# revision 41
# baseline (speedup 1.0000x reference)
"""Conv2D (N=32, Cin=128, 56x56 -> Cout=256, 3x3, pad 1, stride 1) on 8 Trainium2
NeuronCores.

Strategy: data-parallel over batch (4 images per core), conv lowered to 9
shifted matmuls (one per filter tap) accumulating in PSUM over the
Cin=128-partition contraction dim.  Cout=256 is handled as 2 halves of 128
output partitions.

Default mode "fp16v2" (HW-measured 115us vs the 129.7us fp32r baseline):

- fp16 operands: 1 col/cycle on the PE like fp32r, but LDWEIGHTS gets the
  FWL fast path and is fully hidden behind matmuls by the PE's pull-ahead
  (fp32r's self-loading matmuls cost ~+25ns each).  scale-rel err ~4e-4
  (tolerance 2e-2).
- moving operand is a 3D AP [128cin, 8 rows, 56 cols] (row stride 58 over
  the host-padded image): N=448 with no junk columns; the AP row-walk
  measures ~0.5 cyc/row, so this beats the contiguous 464-wide span.
  Warm cadence measures 191ns/MM vs the 189ns floor (448/2.4GHz + NX).
- input is host-pre-sliced into 11-row strips (2-row halo duplicated,
  strip-major DRAM layout) so every strip DMA is one contiguous 163KB
  block: partition-strided DMAs measure ~4us completion latency vs ~2us
  contiguous, and the head is latency-bound.
- weights are tap-major in DRAM; 9 per-tap contiguous 64KB transfers are
  interleaved across both HWDGE rings (taps 0-4 + bias on scalar, taps
  5-8 on sync after strip 0) ordered by when the first 9-tap group
  consumes them.
- NWARM warmup matmuls on a zeroed SBUF tile run while the head DMAs are
  in flight so the PE's HAM clock gate is at 8/8 (2.4GHz) when the real
  stream starts (cold MMs run at 1.2GHz; warming on real work costs ~2us).
- output stays fp16 and fully dense [COUT, 56*56] per image, one
  contiguous 896B span per partition per store on the scalar ring (the
  host upcasts to fp32).  gpsimd/SWDGE is never used (slow per
  descriptor); drains (bias-add PSUM->SBUF) all on vector.

Exec-time anatomy at 115us: ~7.2us fixed TileContext preamble (cross-
engine barriers + instruction load), ~4.3us head DMA latency (hidden
under warmups), 96.5us gapless warm MM stream (504 matmuls), ~5us tail
(last drain + store + end-of-kernel ceremony).
"""

import os
import sys

import numpy as np

sys.path.insert(0, "/opt/trn_rl_repo")

import concourse.tile as tile
from concourse import bacc, mybir

N, CIN, H, W = 32, 128, 56, 56
COUT, KH, KW = 256, 3, 3
NCORES = 8
NPER = N // NCORES  # images per core
HP, WP = H + 3, W + 2  # padded spatial dims (1 top + 2 bottom, 1 left + 1 right)
FLAT = HP * WP  # 3422 padded pixels per image per cin
RB = 8  # output rows per PSUM chunk
NRB = H // RB  # 7 row-blocks per image
CHUNK = RB * WP  # 464 <= 512 fp32 PSUM bank limit
STRIP_ROWS = RB + KH  # 11 padded rows per input strip (8 + 2 halo + 1 overread)
STRIP = STRIP_ROWS * WP  # 638
NTAP = KH * KW

MM_MODE = os.environ.get("CONV_MM_MODE", "fp16v2")

_CACHE = {}


def _build(mm_mode):
    if mm_mode == "fp16v2":
        return _build_v2()
    f32 = mybir.dt.float32
    in_dt = {
        "fp32": f32,
        "fp32r": mybir.dt.float32r,
        "bf16": mybir.dt.bfloat16,
        "fp16": mybir.dt.float16,
    }[mm_mode]

    nc = bacc.Bacc(None, target_bir_lowering=False)
    xp_d = nc.declare_dram_parameter("xp", [NPER, CIN, FLAT], in_dt, isOutput=False)
    w_d = nc.declare_dram_parameter("w", [CIN, NTAP, COUT], in_dt, isOutput=False)
    b_d = nc.declare_dram_parameter("b", [CIN, 2], f32, isOutput=False)
    y_d = nc.declare_dram_parameter("y", [NPER, COUT, H, W], f32, isOutput=True)

    with tile.TileContext(nc) as tc:
        with (
            tc.tile_pool(name="xin", bufs=16) as xpool,
            tc.tile_pool(name="wgt", bufs=1) as wpool,
            tc.tile_pool(name="bias", bufs=1) as bpool,
            tc.tile_pool(name="out", bufs=8) as opool,
            tc.tile_pool(name="ps", bufs=8, space="PSUM") as pspool,
        ):
            # one tile per tap so an MM only waits on its own tap's DMA
            w_taps = []
            for tap in range(NTAP):
                wt = wpool.tile([CIN, COUT], in_dt, tag=f"w{tap}")
                nc.scalar.dma_start(out=wt[:], in_=w_d[:, tap, :])
                w_taps.append(wt)
            b_sb = bpool.tile([CIN, 2], f32)
            nc.scalar.dma_start(out=b_sb[:], in_=b_d[:, :])

            for i in range(NPER):
                for rb in range(NRB):
                    xt = xpool.tile([CIN, STRIP], in_dt, tag="xs")
                    r0 = rb * RB
                    nc.sync.dma_start(
                        out=xt[:], in_=xp_d[i, :, r0 * WP : r0 * WP + STRIP]
                    )
                    for half in range(2):
                        ps = pspool.tile([128, RB, WP], f32)
                        for tap in range(NTAP):
                            kh, kw = divmod(tap, KW)
                            off = kh * WP + kw
                            nc.tensor.matmul(
                                ps[:],
                                w_taps[tap][:, half * 128 : half * 128 + 128],
                                xt[:, off : off + CHUNK],
                                start=(tap == 0),
                                stop=(tap == NTAP - 1),
                            )
                        ot = opool.tile([128, RB, WP], f32)
                        nc.vector.tensor_scalar_add(
                            ot[:], ps[:], b_sb[:, half : half + 1]
                        )
                        nc.scalar.dma_start(
                            out=y_d[i, half * 128 : half * 128 + 128, r0 : r0 + RB, :],
                            in_=ot[:, :, 0:W],
                        )
    nc.finalize()
    return nc


NWARM = int(os.environ.get("CONV_NWARM", "30"))
N448 = os.environ.get("CONV_N448", "1") == "1"


def _build_v2():
    """fp16 matmuls + head/tail fixes over the fp32r baseline:

    - input strips are pre-sliced on the host (halo duplicated) so every
      strip DMA is ONE fully contiguous 163 KB block: strided 128x1276B
      transfers were measured at ~4us completion latency vs <2us
      contiguous.
    - tap-0 weights are duplicated into their own contiguous DRAM blob
      (w0) so the first LDWEIGHTS unblocks ~2us earlier; taps 1-8 come
      as a single contiguous-per-partition transfer.
    - y is stored fp16 WITH the 2 junk columns ([COUT, 56*58] flat) so
      each row-block store is one contiguous 928B span per partition;
      the host strips the junk and upcasts.
    - NWARM warmup matmuls on a zeroed SBUF tile run while the first
      DMAs are in flight, so the PE's HAM clock-gate is at K=8/8 (2.4
      GHz) when the real stream starts instead of warming up on it.
    - ring assignment: strips alone on sync, weights+bias+outputs on
      scalar, drains on vector, gpsimd (slow SWDGE) unused.
    """
    f16 = mybir.dt.float16
    f32 = mybir.dt.float32
    # with CONV_N448, the moving AP is 3D [8 rows x 56 cols] (stride 58):
    # 16 fewer junk columns per matmul, fully dense H*W output
    cw = W if N448 else WP
    chunk = RB * cw

    nc = bacc.Bacc(None, target_bir_lowering=False)
    xp_d = nc.declare_dram_parameter(
        "xp", [NPER, NRB, CIN, STRIP], f16, isOutput=False
    )
    # tap-major so every per-tap transfer is one contiguous 64KB block
    w_d = nc.declare_dram_parameter("w", [NTAP, CIN, COUT], f16, isOutput=False)
    b_d = nc.declare_dram_parameter("b", [CIN, 2], f32, isOutput=False)
    y_d = nc.declare_dram_parameter("y", [NPER, COUT, H * cw], f16, isOutput=True)

    with tile.TileContext(nc) as tc:
        with (
            tc.tile_pool(name="xin", bufs=16) as xpool,
            tc.tile_pool(name="wgt", bufs=1) as wpool,
            tc.tile_pool(name="bias", bufs=1) as bpool,
            tc.tile_pool(name="out", bufs=8) as opool,
            tc.tile_pool(name="ps", bufs=8, space="PSUM") as pspool,
        ):
            # per-tap contiguous weight transfers interleaved across both
            # HWDGE rings in consumption order: the stream needs tap t of
            # its first 9-tap group ~191ns after tap t-1, so early taps
            # must land first; the long transfers are the head's pole.
            wall = wpool.tile([CIN, NTAP, COUT], f16, tag="wall")
            for t in range(5):
                nc.scalar.dma_start(out=wall[:, t, :], in_=w_d[t, :, :])
            b_sb = bpool.tile([CIN, 2], f32)
            nc.scalar.dma_start(out=b_sb[:], in_=b_d[:, :])

            strip_tiles = {}

            def get_strip(i, rb):
                if (i, rb) not in strip_tiles:
                    if N448:
                        xt = xpool.tile([CIN, STRIP_ROWS, WP], f16, tag="xs", name="xt")
                    else:
                        xt = xpool.tile([CIN, STRIP], f16, tag="xs", name="xt")
                    nc.sync.dma_start(out=xt[:], in_=xp_d[i, rb, :, :])
                    strip_tiles[(i, rb)] = xt
                return strip_tiles[(i, rb)]

            get_strip(0, 0)
            for t in range(5, NTAP):
                nc.sync.dma_start(out=wall[:, t, :], in_=w_d[t, :, :])

            if NWARM:
                wz = wpool.tile([CIN, 128], f16, tag="wz")
                nc.vector.memset(wz[:], 0.0)
                pwz = pspool.tile([128, 128], f32, bufs=1)
                for _ in range(NWARM):
                    nc.tensor.matmul(pwz[:], wz[:], wz[:], start=True, stop=True)

            for i in range(NPER):
                for rb in range(NRB):
                    xt = get_strip(i, rb)
                    for half in range(2):
                        ps = pspool.tile([128, RB, cw], f32, bufs=7)
                        for tap in range(NTAP):
                            kh, kw = divmod(tap, KW)
                            if N448:
                                rhs = xt[:, kh : kh + RB, kw : kw + W]
                            else:
                                off = kh * WP + kw
                                rhs = xt[:, off : off + chunk]
                            nc.tensor.matmul(
                                ps[:],
                                wall[:, tap, half * 128 : half * 128 + 128],
                                rhs,
                                start=(tap == 0),
                                stop=(tap == NTAP - 1),
                            )
                        ot = opool.tile([128, RB, cw], f16)
                        ch = slice(half * 128, half * 128 + 128)
                        nc.vector.tensor_scalar_add(
                            ot[:], ps[:], b_sb[:, half : half + 1]
                        )
                        nc.scalar.dma_start(
                            out=y_d[i, ch, rb * chunk : (rb + 1) * chunk],
                            in_=ot[:],
                        )
    nc.finalize()
    return nc


def get_nc(mm_mode=None):
    mm_mode = mm_mode or MM_MODE
    if mm_mode not in _CACHE:
        _CACHE[mm_mode] = _build(mm_mode)
    return _CACHE[mm_mode]


def _round_fp32r(a):
    """Round fp32 array to the fp32r grid (8-bit exp, 11-bit mantissa, top 20
    bits of the word) with round-to-nearest so the PE's truncation of the low
    12 bits lands on the nearest representable value."""
    u = np.ascontiguousarray(a, np.float32).view(np.uint32)
    u = u + 0x7FF + ((u >> 12) & 1)
    u &= np.uint32(0xFFFFF000)
    return u.view(np.float32)


def prep_inputs(x, weight, bias, mm_mode=None):
    """Host-side staging: zero-pad x to 59x58 and flatten, retile weights to
    [cin, tap, cout], split per-core input maps."""
    mm_mode = mm_mode or MM_MODE
    x = np.asarray(x, np.float32)
    weight = np.asarray(weight, np.float32)
    bias = np.asarray(bias, np.float32)

    xp = np.zeros((N, CIN, HP, WP), np.float32)
    xp[:, :, 1 : H + 1, 1 : W + 1] = x
    # [cout, cin, kh, kw] -> [cin, tap, cout]
    w_prep = np.ascontiguousarray(weight.transpose(1, 2, 3, 0).reshape(CIN, NTAP, COUT))
    if mm_mode == "bf16":
        import ml_dtypes

        xp = xp.astype(ml_dtypes.bfloat16)
        w_prep = w_prep.astype(ml_dtypes.bfloat16)
    elif mm_mode in ("fp16", "fp16v2"):
        xp = xp.astype(np.float16)
        w_prep = w_prep.astype(np.float16)
    elif mm_mode == "fp32r":
        xp = _round_fp32r(xp)
        w_prep = _round_fp32r(w_prep)
    xp = xp.reshape(N, CIN, FLAT)
    b_prep = np.ascontiguousarray(bias.reshape(2, 128).T.astype(np.float32))

    if mm_mode == "fp16v2":
        # strip-major input: [N, NRB, CIN, STRIP] with the 2-row halo
        # duplicated, so each strip DMA is one contiguous block.
        xs = np.empty((N, NRB, CIN, STRIP), xp.dtype)
        for rb in range(NRB):
            xs[:, rb] = xp[:, :, rb * RB * WP : rb * RB * WP + STRIP]
        w_tap = np.ascontiguousarray(w_prep.transpose(1, 0, 2))
        return [
            {
                "xp": np.ascontiguousarray(xs[c * NPER : (c + 1) * NPER]),
                "w": w_tap,
                "b": b_prep,
            }
            for c in range(NCORES)
        ]

    return [
        {
            "xp": np.ascontiguousarray(xp[c * NPER : (c + 1) * NPER]),
            "w": w_prep,
            "b": b_prep,
        }
        for c in range(NCORES)
    ]


def kernel(x, weight, bias, mm_mode=None, trace=False, tmpdir=None):
    from concourse.bass_utils import run_bass_kernel_spmd

    nc = get_nc(mm_mode)
    in_maps = prep_inputs(x, weight, bias, mm_mode)
    res = run_bass_kernel_spmd(
        nc, in_maps, list(range(NCORES)), trace=trace, tmpdir=tmpdir
    )
    out = np.concatenate([r["y"] for r in res.results], axis=0)
    out = _post(out)
    if trace:
        kernel.last_results = res
    return out


def _post(y):
    """Device y -> full fp32 [n, COUT, H, W] (strip junk cols, upcast)."""
    y = np.asarray(y)
    n = y.shape[0]
    if y.ndim == 3 and y.shape[2] == H * WP:
        y = y.reshape(n, COUT, H, WP)[:, :, :, :W]
    else:
        y = y.reshape(n, COUT, H, W)
    return np.ascontiguousarray(y, dtype=np.float32)



# revision 44
# speedup vs baseline: 1.0096x; 1.0096x over previous
"""Conv2D (N=32, Cin=128, 56x56 -> Cout=256, 3x3, pad 1, stride 1) on 8 Trainium2
NeuronCores.

Strategy: data-parallel over batch (4 images per core), conv lowered to 9
shifted matmuls (one per filter tap) accumulating in PSUM over the
Cin=128-partition contraction dim.  Cout=256 is handled as 2 halves of 128
output partitions.

Default mode "fp16v2" (HW-measured 115us vs the 129.7us fp32r baseline):

- fp16 operands: 1 col/cycle on the PE like fp32r, but LDWEIGHTS gets the
  FWL fast path and is fully hidden behind matmuls by the PE's pull-ahead
  (fp32r's self-loading matmuls cost ~+25ns each).  scale-rel err ~4e-4
  (tolerance 2e-2).
- moving operand is a 3D AP [128cin, 8 rows, 56 cols] (row stride 58 over
  the host-padded image): N=448 with no junk columns; the AP row-walk
  measures ~0.5 cyc/row, so this beats the contiguous 464-wide span.
  Warm cadence measures 191ns/MM vs the 189ns floor (448/2.4GHz + NX).
- input is host-pre-sliced into 11-row strips (2-row halo duplicated,
  strip-major DRAM layout) so every strip DMA is one contiguous 163KB
  block: partition-strided DMAs measure ~4us completion latency vs ~2us
  contiguous, and the head is latency-bound.
- weights are tap-major in DRAM; 9 per-tap contiguous 64KB transfers are
  interleaved across both HWDGE rings (taps 0-4 + bias on scalar, taps
  5-8 on sync after strip 0) ordered by when the first 9-tap group
  consumes them.
- NWARM warmup matmuls on a zeroed SBUF tile run while the head DMAs are
  in flight so the PE's HAM clock gate is at 8/8 (2.4GHz) when the real
  stream starts (cold MMs run at 1.2GHz; warming on real work costs ~2us).
- output stays fp16 and fully dense [COUT, 56*56] per image, one
  contiguous 896B span per partition per store on the scalar ring (the
  host upcasts to fp32).  gpsimd/SWDGE is never used (slow per
  descriptor); drains (bias-add PSUM->SBUF) all on vector.

Exec-time anatomy at 115us: ~7.2us fixed TileContext preamble (cross-
engine barriers + instruction load), ~4.3us head DMA latency (hidden
under warmups), 96.5us gapless warm MM stream (504 matmuls), ~5us tail
(last drain + store + end-of-kernel ceremony).
"""

import os
import sys

import numpy as np

sys.path.insert(0, "/opt/trn_rl_repo")

import concourse.tile as tile
from concourse import bacc, mybir

N, CIN, H, W = 32, 128, 56, 56
COUT, KH, KW = 256, 3, 3
NCORES = 8
NPER = N // NCORES  # images per core
HP, WP = H + 3, W + 2  # padded spatial dims (1 top + 2 bottom, 1 left + 1 right)
FLAT = HP * WP  # 3422 padded pixels per image per cin
RB = 8  # output rows per PSUM chunk
NRB = H // RB  # 7 row-blocks per image
CHUNK = RB * WP  # 464 <= 512 fp32 PSUM bank limit
STRIP_ROWS = RB + KH  # 11 padded rows per input strip (8 + 2 halo + 1 overread)
STRIP = STRIP_ROWS * WP  # 638
NTAP = KH * KW

MM_MODE = os.environ.get("CONV_MM_MODE", "fp16v2")

_CACHE = {}


def _build(mm_mode):
    if mm_mode == "fp16v2":
        return _build_v2()
    f32 = mybir.dt.float32
    in_dt = {
        "fp32": f32,
        "fp32r": mybir.dt.float32r,
        "bf16": mybir.dt.bfloat16,
        "fp16": mybir.dt.float16,
    }[mm_mode]

    nc = bacc.Bacc(None, target_bir_lowering=False)
    xp_d = nc.declare_dram_parameter("xp", [NPER, CIN, FLAT], in_dt, isOutput=False)
    w_d = nc.declare_dram_parameter("w", [CIN, NTAP, COUT], in_dt, isOutput=False)
    b_d = nc.declare_dram_parameter("b", [CIN, 2], f32, isOutput=False)
    y_d = nc.declare_dram_parameter("y", [NPER, COUT, H, W], f32, isOutput=True)

    with tile.TileContext(nc) as tc:
        with (
            tc.tile_pool(name="xin", bufs=16) as xpool,
            tc.tile_pool(name="wgt", bufs=1) as wpool,
            tc.tile_pool(name="bias", bufs=1) as bpool,
            tc.tile_pool(name="out", bufs=8) as opool,
            tc.tile_pool(name="ps", bufs=8, space="PSUM") as pspool,
        ):
            # one tile per tap so an MM only waits on its own tap's DMA
            w_taps = []
            for tap in range(NTAP):
                wt = wpool.tile([CIN, COUT], in_dt, tag=f"w{tap}")
                nc.scalar.dma_start(out=wt[:], in_=w_d[:, tap, :])
                w_taps.append(wt)
            b_sb = bpool.tile([CIN, 2], f32)
            nc.scalar.dma_start(out=b_sb[:], in_=b_d[:, :])

            for i in range(NPER):
                for rb in range(NRB):
                    xt = xpool.tile([CIN, STRIP], in_dt, tag="xs")
                    r0 = rb * RB
                    nc.sync.dma_start(
                        out=xt[:], in_=xp_d[i, :, r0 * WP : r0 * WP + STRIP]
                    )
                    for half in range(2):
                        ps = pspool.tile([128, RB, WP], f32)
                        for tap in range(NTAP):
                            kh, kw = divmod(tap, KW)
                            off = kh * WP + kw
                            nc.tensor.matmul(
                                ps[:],
                                w_taps[tap][:, half * 128 : half * 128 + 128],
                                xt[:, off : off + CHUNK],
                                start=(tap == 0),
                                stop=(tap == NTAP - 1),
                            )
                        ot = opool.tile([128, RB, WP], f32)
                        nc.vector.tensor_scalar_add(
                            ot[:], ps[:], b_sb[:, half : half + 1]
                        )
                        nc.scalar.dma_start(
                            out=y_d[i, half * 128 : half * 128 + 128, r0 : r0 + RB, :],
                            in_=ot[:, :, 0:W],
                        )
    nc.finalize()
    return nc


NWARM = int(os.environ.get("CONV_NWARM", "38"))
N448 = os.environ.get("CONV_N448", "1") == "1"


def _build_v2():
    """fp16 matmuls + head/tail fixes over the fp32r baseline; see the
    module docstring for the full design rationale."""
    f16 = mybir.dt.float16
    f32 = mybir.dt.float32
    # with CONV_N448, the moving AP is 3D [8 rows x 56 cols] (stride 58):
    # 16 fewer junk columns per matmul, fully dense H*W output
    cw = W if N448 else WP
    chunk = RB * cw

    nc = bacc.Bacc(None, target_bir_lowering=False)
    xp_d = nc.declare_dram_parameter(
        "xp", [NPER, NRB, CIN, STRIP], f16, isOutput=False
    )
    # tap-major so every per-tap transfer is one contiguous 64KB block
    w_d = nc.declare_dram_parameter("w", [NTAP, CIN, COUT], f16, isOutput=False)
    b_d = nc.declare_dram_parameter("b", [CIN, 2], f32, isOutput=False)
    y_d = nc.declare_dram_parameter("y", [NPER, COUT, H * cw], f16, isOutput=True)

    with tile.TileContext(nc) as tc:
        with (
            tc.tile_pool(name="xin", bufs=16) as xpool,
            tc.tile_pool(name="wgt", bufs=1) as wpool,
            tc.tile_pool(name="bias", bufs=1) as bpool,
            tc.tile_pool(name="out", bufs=8) as opool,
            tc.tile_pool(name="ps", bufs=8, space="PSUM") as pspool,
        ):
            # per-tap contiguous weight transfers interleaved across both
            # HWDGE rings in consumption order: the stream needs tap t of
            # its first 9-tap group ~191ns after tap t-1, so early taps
            # must land first; the long transfers are the head's pole.
            wall = wpool.tile([CIN, NTAP, COUT], f16, tag="wall")
            for t in range(5):
                nc.scalar.dma_start(out=wall[:, t, :], in_=w_d[t, :, :])
            b_sb = bpool.tile([CIN, 2], f32)
            nc.scalar.dma_start(out=b_sb[:], in_=b_d[:, :])

            strip_tiles = {}

            def get_strip(i, rb):
                if (i, rb) not in strip_tiles:
                    if N448:
                        xt = xpool.tile([CIN, STRIP_ROWS, WP], f16, tag="xs", name="xt")
                    else:
                        xt = xpool.tile([CIN, STRIP], f16, tag="xs", name="xt")
                    nc.sync.dma_start(out=xt[:], in_=xp_d[i, rb, :, :])
                    strip_tiles[(i, rb)] = xt
                return strip_tiles[(i, rb)]

            get_strip(0, 0)
            nc.sync.dma_start(out=wall[:, 5, :], in_=w_d[5, :, :])
            nc.sync.dma_start(out=wall[:, 6, :], in_=w_d[6, :, :])
            get_strip(0, 1)
            nc.sync.dma_start(out=wall[:, 7, :], in_=w_d[7, :, :])
            nc.sync.dma_start(out=wall[:, 8, :], in_=w_d[8, :, :])

            if NWARM:
                wz = wpool.tile([CIN, 128], f16, tag="wz")
                nc.vector.memset(wz[:], 0.0)
                pwz = pspool.tile([128, 128], f32, bufs=1)
                for _ in range(NWARM):
                    nc.tensor.matmul(pwz[:], wz[:], wz[:], start=True, stop=True)

            for i in range(NPER):
                for rb in range(NRB):
                    xt = get_strip(i, rb)
                    for half in range(2):
                        ps = pspool.tile([128, RB, cw], f32, bufs=7)
                        for tap in range(NTAP):
                            kh, kw = divmod(tap, KW)
                            if N448:
                                rhs = xt[:, kh : kh + RB, kw : kw + W]
                            else:
                                off = kh * WP + kw
                                rhs = xt[:, off : off + chunk]
                            nc.tensor.matmul(
                                ps[:],
                                wall[:, tap, half * 128 : half * 128 + 128],
                                rhs,
                                start=(tap == 0),
                                stop=(tap == NTAP - 1),
                            )
                        ot = opool.tile([128, RB, cw], f16)
                        ch = slice(half * 128, half * 128 + 128)
                        nc.vector.tensor_scalar_add(
                            ot[:], ps[:], b_sb[:, half : half + 1]
                        )
                        nc.scalar.dma_start(
                            out=y_d[i, ch, rb * chunk : (rb + 1) * chunk],
                            in_=ot[:],
                        )
    nc.finalize()
    return nc


def get_nc(mm_mode=None):
    mm_mode = mm_mode or MM_MODE
    if mm_mode not in _CACHE:
        _CACHE[mm_mode] = _build(mm_mode)
    return _CACHE[mm_mode]


def _round_fp32r(a):
    """Round fp32 array to the fp32r grid (8-bit exp, 11-bit mantissa, top 20
    bits of the word) with round-to-nearest so the PE's truncation of the low
    12 bits lands on the nearest representable value."""
    u = np.ascontiguousarray(a, np.float32).view(np.uint32)
    u = u + 0x7FF + ((u >> 12) & 1)
    u &= np.uint32(0xFFFFF000)
    return u.view(np.float32)


def prep_inputs(x, weight, bias, mm_mode=None):
    """Host-side staging: zero-pad x to 59x58 and flatten, retile weights to
    [cin, tap, cout], split per-core input maps."""
    mm_mode = mm_mode or MM_MODE
    x = np.asarray(x, np.float32)
    weight = np.asarray(weight, np.float32)
    bias = np.asarray(bias, np.float32)

    xp = np.zeros((N, CIN, HP, WP), np.float32)
    xp[:, :, 1 : H + 1, 1 : W + 1] = x
    # [cout, cin, kh, kw] -> [cin, tap, cout]
    w_prep = np.ascontiguousarray(weight.transpose(1, 2, 3, 0).reshape(CIN, NTAP, COUT))
    if mm_mode == "bf16":
        import ml_dtypes

        xp = xp.astype(ml_dtypes.bfloat16)
        w_prep = w_prep.astype(ml_dtypes.bfloat16)
    elif mm_mode in ("fp16", "fp16v2"):
        xp = xp.astype(np.float16)
        w_prep = w_prep.astype(np.float16)
    elif mm_mode == "fp32r":
        xp = _round_fp32r(xp)
        w_prep = _round_fp32r(w_prep)
    xp = xp.reshape(N, CIN, FLAT)
    b_prep = np.ascontiguousarray(bias.reshape(2, 128).T.astype(np.float32))

    if mm_mode == "fp16v2":
        # strip-major input: [N, NRB, CIN, STRIP] with the 2-row halo
        # duplicated, so each strip DMA is one contiguous block.
        xs = np.empty((N, NRB, CIN, STRIP), xp.dtype)
        for rb in range(NRB):
            xs[:, rb] = xp[:, :, rb * RB * WP : rb * RB * WP + STRIP]
        w_tap = np.ascontiguousarray(w_prep.transpose(1, 0, 2))
        return [
            {
                "xp": np.ascontiguousarray(xs[c * NPER : (c + 1) * NPER]),
                "w": w_tap,
                "b": b_prep,
            }
            for c in range(NCORES)
        ]

    return [
        {
            "xp": np.ascontiguousarray(xp[c * NPER : (c + 1) * NPER]),
            "w": w_prep,
            "b": b_prep,
        }
        for c in range(NCORES)
    ]


def kernel(x, weight, bias, mm_mode=None, trace=False, tmpdir=None):
    from concourse.bass_utils import run_bass_kernel_spmd

    nc = get_nc(mm_mode)
    in_maps = prep_inputs(x, weight, bias, mm_mode)
    res = run_bass_kernel_spmd(
        nc, in_maps, list(range(NCORES)), trace=trace, tmpdir=tmpdir
    )
    out = np.concatenate([r["y"] for r in res.results], axis=0)
    out = _post(out)
    if trace:
        kernel.last_results = res
    return out


def _post(y):
    """Device y -> full fp32 [n, COUT, H, W] (strip junk cols, upcast)."""
    y = np.asarray(y)
    n = y.shape[0]
    if y.ndim == 3 and y.shape[2] == H * WP:
        y = y.reshape(n, COUT, H, WP)[:, :, :, :W]
    else:
        y = y.reshape(n, COUT, H, W)
    return np.ascontiguousarray(y, dtype=np.float32)



# revision 49
# speedup vs baseline: 1.0143x; 1.0046x over previous
"""Conv2D (N=32, Cin=128, 56x56 -> Cout=256, 3x3, pad 1, stride 1) on 8 Trainium2
NeuronCores.

Strategy: data-parallel over batch (4 images per core), conv lowered to 9
shifted matmuls (one per filter tap) accumulating in PSUM over the
Cin=128-partition contraction dim.  Cout=256 is handled as 2 halves of 128
output partitions.

Default mode "fp16v2" (HW-measured 115us vs the 129.7us fp32r baseline):

- fp16 operands: 1 col/cycle on the PE like fp32r, but LDWEIGHTS gets the
  FWL fast path and is fully hidden behind matmuls by the PE's pull-ahead
  (fp32r's self-loading matmuls cost ~+25ns each).  scale-rel err ~4e-4
  (tolerance 2e-2).
- moving operand is a 3D AP [128cin, 8 rows, 56 cols] (row stride 58 over
  the host-padded image): N=448 with no junk columns; the AP row-walk
  measures ~0.5 cyc/row, so this beats the contiguous 464-wide span.
  Warm cadence measures 191ns/MM vs the 189ns floor (448/2.4GHz + NX).
- input is host-pre-sliced into 11-row strips (2-row halo duplicated,
  strip-major DRAM layout) so every strip DMA is one contiguous 163KB
  block: partition-strided DMAs measure ~4us completion latency vs ~2us
  contiguous, and the head is latency-bound.
- weights are tap-major in DRAM; 9 per-tap contiguous 64KB transfers are
  interleaved across both HWDGE rings (taps 0-4 + bias on scalar, taps
  5-8 on sync after strip 0) ordered by when the first 9-tap group
  consumes them.
- NWARM warmup matmuls on a zeroed SBUF tile run while the head DMAs are
  in flight so the PE's HAM clock gate is at 8/8 (2.4GHz) when the real
  stream starts (cold MMs run at 1.2GHz; warming on real work costs ~2us).
- output stays fp16 and fully dense [COUT, 56*56] per image, one
  contiguous 896B span per partition per store on the scalar ring (the
  host upcasts to fp32).  gpsimd/SWDGE is never used (slow per
  descriptor); drains (bias-add PSUM->SBUF) all on vector.

Exec-time anatomy at 115us: ~7.2us fixed TileContext preamble (cross-
engine barriers + instruction load), ~4.3us head DMA latency (hidden
under warmups), 96.5us gapless warm MM stream (504 matmuls), ~5us tail
(last drain + store + end-of-kernel ceremony).
"""

import os
import sys

import numpy as np

sys.path.insert(0, "/opt/trn_rl_repo")

import concourse.tile as tile
from concourse import bacc, mybir

N, CIN, H, W = 32, 128, 56, 56
COUT, KH, KW = 256, 3, 3
NCORES = 8
NPER = N // NCORES  # images per core
HP, WP = H + 3, W + 2  # padded spatial dims (1 top + 2 bottom, 1 left + 1 right)
FLAT = HP * WP  # 3422 padded pixels per image per cin
RB = 8  # output rows per PSUM chunk
NRB = H // RB  # 7 row-blocks per image
CHUNK = RB * WP  # 464 <= 512 fp32 PSUM bank limit
STRIP_ROWS = RB + KH  # 11 padded rows per input strip (8 + 2 halo + 1 overread)
STRIP = STRIP_ROWS * WP  # 638
NTAP = KH * KW

MM_MODE = os.environ.get("CONV_MM_MODE", "fp16v2")

_CACHE = {}


def _build(mm_mode):
    if mm_mode == "fp16v2":
        return _build_v2()
    f32 = mybir.dt.float32
    in_dt = {
        "fp32": f32,
        "fp32r": mybir.dt.float32r,
        "bf16": mybir.dt.bfloat16,
        "fp16": mybir.dt.float16,
    }[mm_mode]

    nc = bacc.Bacc(None, target_bir_lowering=False)
    xp_d = nc.declare_dram_parameter("xp", [NPER, CIN, FLAT], in_dt, isOutput=False)
    w_d = nc.declare_dram_parameter("w", [CIN, NTAP, COUT], in_dt, isOutput=False)
    b_d = nc.declare_dram_parameter("b", [CIN, 2], f32, isOutput=False)
    y_d = nc.declare_dram_parameter("y", [NPER, COUT, H, W], f32, isOutput=True)

    with tile.TileContext(nc) as tc:
        with (
            tc.tile_pool(name="xin", bufs=16) as xpool,
            tc.tile_pool(name="wgt", bufs=1) as wpool,
            tc.tile_pool(name="bias", bufs=1) as bpool,
            tc.tile_pool(name="out", bufs=8) as opool,
            tc.tile_pool(name="ps", bufs=8, space="PSUM") as pspool,
        ):
            # one tile per tap so an MM only waits on its own tap's DMA
            w_taps = []
            for tap in range(NTAP):
                wt = wpool.tile([CIN, COUT], in_dt, tag=f"w{tap}")
                nc.scalar.dma_start(out=wt[:], in_=w_d[:, tap, :])
                w_taps.append(wt)
            b_sb = bpool.tile([CIN, 2], f32)
            nc.scalar.dma_start(out=b_sb[:], in_=b_d[:, :])

            for i in range(NPER):
                for rb in range(NRB):
                    xt = xpool.tile([CIN, STRIP], in_dt, tag="xs")
                    r0 = rb * RB
                    nc.sync.dma_start(
                        out=xt[:], in_=xp_d[i, :, r0 * WP : r0 * WP + STRIP]
                    )
                    for half in range(2):
                        ps = pspool.tile([128, RB, WP], f32)
                        for tap in range(NTAP):
                            kh, kw = divmod(tap, KW)
                            off = kh * WP + kw
                            nc.tensor.matmul(
                                ps[:],
                                w_taps[tap][:, half * 128 : half * 128 + 128],
                                xt[:, off : off + CHUNK],
                                start=(tap == 0),
                                stop=(tap == NTAP - 1),
                            )
                        ot = opool.tile([128, RB, WP], f32)
                        nc.vector.tensor_scalar_add(
                            ot[:], ps[:], b_sb[:, half : half + 1]
                        )
                        nc.scalar.dma_start(
                            out=y_d[i, half * 128 : half * 128 + 128, r0 : r0 + RB, :],
                            in_=ot[:, :, 0:W],
                        )
    nc.finalize()
    return nc


NWARM = int(os.environ.get("CONV_NWARM", "38"))
N448 = os.environ.get("CONV_N448", "1") == "1"
# N448 taps only read strip rows kh..kh+7 (kh<=2), so 10 rows suffice;
# the flat-464 path over-reads into row 10 and needs 11.
SROWS = 10 if N448 else STRIP_ROWS
SLEN = SROWS * WP


def _build_v2():
    """fp16 matmuls + head/tail fixes over the fp32r baseline; see the
    module docstring for the full design rationale."""
    f16 = mybir.dt.float16
    f32 = mybir.dt.float32
    # with CONV_N448, the moving AP is 3D [8 rows x 56 cols] (stride 58):
    # 16 fewer junk columns per matmul, fully dense H*W output
    cw = W if N448 else WP
    chunk = RB * cw

    nc = bacc.Bacc(None, target_bir_lowering=False)
    xp_d = nc.declare_dram_parameter(
        "xp", [NPER, NRB, CIN, SLEN], f16, isOutput=False
    )
    # tap-major so every per-tap transfer is one contiguous 64KB block
    w_d = nc.declare_dram_parameter("w", [NTAP, CIN, COUT], f16, isOutput=False)
    b_d = nc.declare_dram_parameter("b", [CIN, 2], f32, isOutput=False)
    y_d = nc.declare_dram_parameter("y", [NPER, COUT, H * cw], f16, isOutput=True)

    with tile.TileContext(nc) as tc:
        with (
            tc.tile_pool(name="xin", bufs=16) as xpool,
            tc.tile_pool(name="wgt", bufs=1) as wpool,
            tc.tile_pool(name="bias", bufs=1) as bpool,
            tc.tile_pool(name="out", bufs=8) as opool,
            tc.tile_pool(name="ps", bufs=8, space="PSUM") as pspool,
        ):
            # per-tap contiguous weight transfers interleaved across both
            # HWDGE rings in consumption order: the stream needs tap t of
            # its first 9-tap group ~191ns after tap t-1, so early taps
            # must land first; the long transfers are the head's pole.
            wall = wpool.tile([CIN, NTAP, COUT], f16, tag="wall")
            for t in range(5):
                nc.scalar.dma_start(out=wall[:, t, :], in_=w_d[t, :, :])
            b_sb = bpool.tile([CIN, 2], f32)
            nc.scalar.dma_start(out=b_sb[:], in_=b_d[:, :])

            strip_tiles = {}

            def get_strip(i, rb):
                if (i, rb) not in strip_tiles:
                    if N448:
                        xt = xpool.tile([CIN, SROWS, WP], f16, tag="xs", name="xt")
                    else:
                        xt = xpool.tile([CIN, SLEN], f16, tag="xs", name="xt")
                    nc.sync.dma_start(out=xt[:], in_=xp_d[i, rb, :, :])
                    strip_tiles[(i, rb)] = xt
                return strip_tiles[(i, rb)]

            get_strip(0, 0)
            nc.sync.dma_start(out=wall[:, 5, :], in_=w_d[5, :, :])
            nc.sync.dma_start(out=wall[:, 6, :], in_=w_d[6, :, :])
            get_strip(0, 1)
            nc.sync.dma_start(out=wall[:, 7, :], in_=w_d[7, :, :])
            nc.sync.dma_start(out=wall[:, 8, :], in_=w_d[8, :, :])

            if NWARM:
                wz = wpool.tile([CIN, 128], f16, tag="wz")
                nc.vector.memset(wz[:], 0.0)
                pwz = pspool.tile([128, 128], f32, bufs=1)
                for _ in range(NWARM):
                    nc.tensor.matmul(pwz[:], wz[:], wz[:], start=True, stop=True)

            for i in range(NPER):
                for rb in range(NRB):
                    xt = get_strip(i, rb)
                    for half in range(2):
                        ps = pspool.tile([128, RB, cw], f32, bufs=7)
                        for tap in range(NTAP):
                            kh, kw = divmod(tap, KW)
                            if N448:
                                rhs = xt[:, kh : kh + RB, kw : kw + W]
                            else:
                                off = kh * WP + kw
                                rhs = xt[:, off : off + chunk]
                            nc.tensor.matmul(
                                ps[:],
                                wall[:, tap, half * 128 : half * 128 + 128],
                                rhs,
                                start=(tap == 0),
                                stop=(tap == NTAP - 1),
                            )
                        ot = opool.tile([128, RB, cw], f16)
                        ch = slice(half * 128, half * 128 + 128)
                        if i == NPER - 1 and rb == NRB - 1 and half == 1:
                            # very last drain: two half-size vector drains,
                            # each store triggered as soon as its half is
                            # ready (shaves ~0.6us off the tail; both
                            # drains stay on vector so nothing serializes
                            # behind DMA triggers on the scalar queue)
                            hb = RB // 2
                            nc.vector.tensor_scalar_add(
                                ot[:, 0:hb, :], ps[:, 0:hb, :], b_sb[:, half : half + 1]
                            )
                            nc.scalar.dma_start(
                                out=y_d[i, ch, rb * chunk : rb * chunk + hb * cw],
                                in_=ot[:, 0:hb, :],
                            )
                            nc.vector.tensor_scalar_add(
                                ot[:, hb:RB, :], ps[:, hb:RB, :], b_sb[:, half : half + 1]
                            )
                            nc.scalar.dma_start(
                                out=y_d[i, ch, rb * chunk + hb * cw : (rb + 1) * chunk],
                                in_=ot[:, hb:RB, :],
                            )
                        else:
                            nc.vector.tensor_scalar_add(
                                ot[:], ps[:], b_sb[:, half : half + 1]
                            )
                            nc.scalar.dma_start(
                                out=y_d[i, ch, rb * chunk : (rb + 1) * chunk],
                                in_=ot[:],
                            )
    nc.finalize()
    return nc


def get_nc(mm_mode=None):
    mm_mode = mm_mode or MM_MODE
    if mm_mode not in _CACHE:
        _CACHE[mm_mode] = _build(mm_mode)
    return _CACHE[mm_mode]


def _round_fp32r(a):
    """Round fp32 array to the fp32r grid (8-bit exp, 11-bit mantissa, top 20
    bits of the word) with round-to-nearest so the PE's truncation of the low
    12 bits lands on the nearest representable value."""
    u = np.ascontiguousarray(a, np.float32).view(np.uint32)
    u = u + 0x7FF + ((u >> 12) & 1)
    u &= np.uint32(0xFFFFF000)
    return u.view(np.float32)


def prep_inputs(x, weight, bias, mm_mode=None):
    """Host-side staging: zero-pad x to 59x58 and flatten, retile weights to
    [cin, tap, cout], split per-core input maps."""
    mm_mode = mm_mode or MM_MODE
    x = np.asarray(x, np.float32)
    weight = np.asarray(weight, np.float32)
    bias = np.asarray(bias, np.float32)

    xp = np.zeros((N, CIN, HP, WP), np.float32)
    xp[:, :, 1 : H + 1, 1 : W + 1] = x
    # [cout, cin, kh, kw] -> [cin, tap, cout]
    w_prep = np.ascontiguousarray(weight.transpose(1, 2, 3, 0).reshape(CIN, NTAP, COUT))
    if mm_mode == "bf16":
        import ml_dtypes

        xp = xp.astype(ml_dtypes.bfloat16)
        w_prep = w_prep.astype(ml_dtypes.bfloat16)
    elif mm_mode in ("fp16", "fp16v2"):
        xp = xp.astype(np.float16)
        w_prep = w_prep.astype(np.float16)
    elif mm_mode == "fp32r":
        xp = _round_fp32r(xp)
        w_prep = _round_fp32r(w_prep)
    xp = xp.reshape(N, CIN, FLAT)
    b_prep = np.ascontiguousarray(bias.reshape(2, 128).T.astype(np.float32))

    if mm_mode == "fp16v2":
        # strip-major input: [N, NRB, CIN, SLEN] with the 2-row halo
        # duplicated, so each strip DMA is one contiguous block.
        xs = np.empty((N, NRB, CIN, SLEN), xp.dtype)
        for rb in range(NRB):
            xs[:, rb] = xp[:, :, rb * RB * WP : rb * RB * WP + SLEN]
        w_tap = np.ascontiguousarray(w_prep.transpose(1, 0, 2))
        return [
            {
                "xp": np.ascontiguousarray(xs[c * NPER : (c + 1) * NPER]),
                "w": w_tap,
                "b": b_prep,
            }
            for c in range(NCORES)
        ]

    return [
        {
            "xp": np.ascontiguousarray(xp[c * NPER : (c + 1) * NPER]),
            "w": w_prep,
            "b": b_prep,
        }
        for c in range(NCORES)
    ]


def kernel(x, weight, bias, mm_mode=None, trace=False, tmpdir=None):
    from concourse.bass_utils import run_bass_kernel_spmd

    nc = get_nc(mm_mode)
    in_maps = prep_inputs(x, weight, bias, mm_mode)
    res = run_bass_kernel_spmd(
        nc, in_maps, list(range(NCORES)), trace=trace, tmpdir=tmpdir
    )
    out = np.concatenate([r["y"] for r in res.results], axis=0)
    out = _post(out)
    if trace:
        kernel.last_results = res
    return out


def _post(y):
    """Device y -> full fp32 [n, COUT, H, W] (strip junk cols, upcast)."""
    y = np.asarray(y)
    n = y.shape[0]
    if y.ndim == 3 and y.shape[2] == H * WP:
        y = y.reshape(n, COUT, H, WP)[:, :, :, :W]
    else:
        y = y.reshape(n, COUT, H, W)
    return np.ascontiguousarray(y, dtype=np.float32)

